# revision 1
# baseline (speedup 1.0000x reference)
"""GQA attention (B=1, S=2048, H=2048, 32 q-heads / 8 kv-heads, hd=64)
on 8 Trainium2 NeuronCores.

Sharding: tensor-parallel over heads. Core c owns q-heads 4c..4c+3 and
kv-head c: wq/wk/wv column shards, wo row shard; each core computes a
full [S, H] partial of the output projection; chunked ReduceScatters
(256 output rows each, overlapped with compute) sum the partials; the
host scatters the per-core slices back together.

Device program (per core), all matmuls fp32r (~bf16 rate, ~1e-4 rel):
  phase AB, pipelined per 1024-column half:
    qT/kT/vT projections (weights host-pretiled for contiguous DMA),
    per-head RMSNorm (ones-block matmul partition sums; rstd via ACT
    Ln -> Exp(-0.5x), table switches batched), RoPE via partition-
    shifted sbuf-sbuf DMA + 3 DVE ops, V transposed on TensorE into
    V_aug with a ones column.
  phase CDE per 512-wide q-chunk:
    scores^T [128 kpos, q] = kT-tile.T @ qT, 2 heads row-packed into
    one [128,1024] psum, P^T = exp(0.125 S^T) on ScalarE (bounded
    scores: no max pass), causal mask on the diagonal 128x128 block,
    attnT_aug [65, q] += V_aug.T @ P^T (ones column -> l),
    normalize by 1/l = Exp(-Ln(l)) broadcast via ones-block matmul,
    o_proj per 128-row tile, ReduceScatter every 256 rows.
"""
import os
import sys

sys.path.insert(0, "/opt/trn_rl_repo")

import numpy as np  # noqa: E402
import concourse.bacc as bacc  # noqa: E402
import concourse.mybir as mybir  # noqa: E402
import concourse.tile as tile  # noqa: E402
from concourse import bass_utils  # noqa: E402

f32 = mybir.dt.float32
f32r = mybir.dt.float32r
bf16 = mybir.dt.bfloat16
AF = mybir.ActivationFunctionType

N_CORES = 8
S = 2048
HID = 2048
HD = 64
ROPE_THETA = 10000.0
RMS_EPS = 1e-6
SCALING = HD ** -0.5              # 0.125
NK = HID // 128                   # 16 contraction tiles
NQC = S // 512                    # 4 q chunks
NKT = S // 128                    # 16 kpos tiles
RS_BF16 = os.environ.get("KRS16", "0") == "1"

_NC_CACHE = None
LAST_RESULTS = None


def _build():
    nc = bacc.Bacc("TRN2", target_bir_lowering=False, debug=False,
                   num_devices=N_CORES)

    def din(name, shape, dt):
        return nc.dram_tensor(name, shape, dt, kind="ExternalInput").ap()

    xT = din("xT", [HID, S], f32r)
    # host-pretiled: row p, col block t = original rows 128t+p
    wq0 = din("wq0", [128, HID], f32r)
    wq1 = din("wq1", [128, HID], f32r)
    wkv = din("wkv", [128, HID], f32r)     # [wv | wk] columns pretiled
    wo0 = din("wo0", [128, S], f32r)
    wo1 = din("wo1", [128, S], f32r)
    cos2 = din("cos2", [128, S], f32)
    ss2 = din("ss2", [128, S], f32)
    ew_q = din("ew_q", [2, 128], f32r)
    ew_k = din("ew_k", [2, 128], f32r)
    e2 = din("e2", [2, 128], f32r)
    e2t = din("e2t", [128, 2], f32r)
    mask = din("mask", [128, 128], f32r)
    ident = din("ident", [64, 64], f32)

    out_rs = nc.dram_tensor("out_rs", [S // N_CORES, S], f32,
                            kind="ExternalOutput").ap()

    rs_dt = bf16 if RS_BF16 else f32

    with tile.TileContext(nc) as tc:
        with tc.tile_pool(name="consts", bufs=1) as cp, \
             tc.tile_pool(name="dram", bufs=1, space="DRAM") as dp:
            c_wq0 = cp.tile([128, HID], f32r, tag="w")
            c_wq1 = cp.tile([128, HID], f32r, tag="w2")
            c_wkv = cp.tile([128, HID], f32r, tag="w3")
            c_wo0 = cp.tile([128, S], f32r, tag="w4")
            c_wo1 = cp.tile([128, S], f32r, tag="w5")
            c_cos = cp.tile([128, S], f32, tag="c1")
            c_ss = cp.tile([128, S], f32, tag="c2")
            c_ewq = cp.tile([2, 128], f32r, tag="c3")
            c_ewk = cp.tile([2, 128], f32r, tag="c4")
            c_e2 = cp.tile([2, 128], f32r, tag="c5")
            c_e2t = cp.tile([128, 2], f32r, tag="c5t")
            c_mask = cp.tile([128, 128], f32r, tag="c6")
            c_id = cp.tile([64, 64], f32, tag="c7")
            c_eps = cp.tile([2, 1], f32, tag="c8")

            # phase-A weights first (contiguous, 8KB rows);
            # wq0 complete first so the first matmul unblocks early
            for dst_t, src_t in ((c_wq0, wq0), (c_wq1, wq1),
                                 (c_wkv, wkv)):
                for h in range(4):
                    hr = slice(32 * h, 32 * h + 32)
                    nc.sync.dma_start(dst_t[hr, :], src_t[hr, :])
            nc.vector.memset(c_eps[:], RMS_EPS)
            nc.sync.dma_start(c_e2t[:], e2t)
            nc.sync.dma_start(c_ewq[:], ew_q)
            nc.sync.dma_start(c_ewk[:], ew_k)
            nc.sync.dma_start(c_id[:], ident)
            nc.sync.dma_start(c_cos[:], cos2)
            nc.sync.dma_start(c_ss[:], ss2)

            qkv = {
                "q0": cp.tile([128, S], f32, tag="q0", name="q0"),
                "q1": cp.tile([128, S], f32, tag="q1", name="q1"),
                "kv": cp.tile([128, S], f32, tag="kv", name="kv"),
            }
            qr0 = cp.tile([128, S], f32r, tag="qr0")
            qr1 = cp.tile([128, S], f32r, tag="qr1")
            krd = cp.tile([128, S], f32r, tag="krd")
            v_aug = cp.tile([128, NKT * (HD + 1)], f32r, tag="vaug")

            attn_raw = [cp.tile([128, S], f32, tag=f"araw{i}",
                                name=f"araw{i}") for i in range(2)]
            l_sb = [cp.tile([2, S], f32, tag=f"l{i}", name=f"l{i}")
                    for i in range(2)]

            partial = dp.tile([S, S], rs_dt)
            rs_out = dp.tile([S // N_CORES, S], rs_dt)

            # ---- Phase A+B pipelined per 1024-col half ----
            with tc.tile_pool(name="xt", bufs=4) as xp, \
                 tc.tile_pool(name="sbB", bufs=2) as sbB, \
                 tc.tile_pool(name="psA", bufs=3, space="PSUM") as psA, \
                 tc.tile_pool(name="psM", bufs=2, space="PSUM") as psM:
                specs = [
                    ("kv", c_ewk, krd, True),
                    ("q0", c_ewq, qr0, False),
                    ("q1", c_ewq, qr1, False),
                ]
                for qh in range(2):
                    hs = slice(1024 * qh, 1024 * qh + 1024)
                    # --- A: projections for this half ---
                    pq = [psA.tile([128, 1024], f32, tag="pa",
                                   name=f"pa{qh}_{j}") for j in range(3)]
                    for t in range(NK):
                        xt = xp.tile([128, 1024], f32r, tag="xt")
                        for h in range(2):
                            hr = slice(64 * h, 64 * h + 64)
                            nc.sync.dma_start(
                                xt[hr, :],
                                xT[128 * t + 64 * h:128 * t + 64 * h + 64,
                                   hs])
                        st = (t == 0)
                        sp = (t == NK - 1)
                        tc_ = slice(128 * t, 128 * (t + 1))
                        for j, w in ((0, c_wq0), (1, c_wq1), (2, c_wkv)):
                            nc.tensor.matmul(pq[j][:, 0:512], w[:, tc_],
                                             xt[:, 0:512],
                                             start=st, stop=sp)
                            nc.tensor.matmul(pq[j][:, 512:1024], w[:, tc_],
                                             xt[:, 512:1024],
                                             start=st, stop=sp)
                    for j, key in ((0, "q0"), (1, "q1"), (2, "kv")):
                        nc.vector.tensor_copy(qkv[key][:, hs], pq[j][:])

                    # --- B: norm + rope for the two 512-chunks ---
                    # stats: Ln batch then Exp batch (2 table switches)
                    lnvs = {}
                    for si, (key, ew, dst, is_kv) in enumerate(specs):
                        src = qkv[key]
                        sq = sbB.tile([128, 1024], f32r, tag="sq",
                                      bufs=2, name=f"sq{qh}_{si}")
                        nc.vector.tensor_mul(sq[:], src[:, hs], src[:, hs])
                        for u in range(2):
                            us = slice(512 * u, 512 * u + 512)
                            pss = psM.tile([2, 512], f32, tag="m",
                                           name=f"ss{qh}_{si}_{u}")
                            nc.tensor.matmul(pss[:], c_e2t[:], sq[:, us],
                                             start=True, stop=True)
                            lnv = sbB.tile([2, 512], f32, tag="lnv",
                                           bufs=6, name=f"lnv{qh}{si}{u}")
                            nc.scalar.activation(lnv[:], pss[:], AF.Ln,
                                                 scale=1.0 / HD,
                                                 bias=c_eps[:])
                            lnvs[(si, u)] = lnv
                    rstds = {}
                    for si in range(3):
                        for u in range(2):
                            rr = sbB.tile([2, 512], f32r, tag="rstdr",
                                          bufs=6, name=f"rr{qh}{si}{u}")
                            nc.scalar.activation(rr[:], lnvs[(si, u)][:],
                                                 AF.Exp, scale=-0.5)
                            rstds[(si, u)] = rr
                    for si, (key, ew, dst, is_kv) in enumerate(specs):
                        src = qkv[key]
                        rows = slice(64, 128) if is_kv else slice(0, 128)
                        nrm = sbB.tile([128, 1024], f32, tag="nrm",
                                       bufs=2, name=f"nrm{qh}_{si}")
                        for u in range(2):
                            cs = slice(1024 * qh + 512 * u,
                                       1024 * qh + 512 * u + 512)
                            us = slice(512 * u, 512 * u + 512)
                            pb = psM.tile([128, 512], f32, tag="m",
                                          name=f"pb{qh}_{si}_{u}")
                            nc.tensor.matmul(pb[:], ew[:],
                                             rstds[(si, u)][:],
                                             start=True, stop=True)
                            nc.vector.tensor_mul(nrm[rows, us],
                                                 src[rows, cs],
                                                 pb[rows, :])
                        # rope
                        sh = sbB.tile([128, 1024], f32, tag="sh",
                                      bufs=2, name=f"sh{qh}_{si}")
                        if is_kv:
                            nc.sync.dma_start(sh[64:96, :], nrm[96:128, :])
                            nc.sync.dma_start(sh[96:128, :], nrm[64:96, :])
                        else:
                            nc.sync.dma_start(sh[0:32, :], nrm[32:64, :])
                            nc.sync.dma_start(sh[32:64, :], nrm[0:32, :])
                            nc.sync.dma_start(sh[64:96, :], nrm[96:128, :])
                            nc.sync.dma_start(sh[96:128, :], nrm[64:96, :])
                        t2 = sbB.tile([128, 1024], f32, tag="sq",
                                      bufs=2, name=f"t2{qh}_{si}")
                        nc.vector.tensor_mul(t2[rows, :], sh[rows, :],
                                             c_ss[rows, hs])
                        t1 = sbB.tile([128, 1024], f32, tag="sh",
                                      bufs=2, name=f"t1{qh}_{si}")
                        nc.vector.tensor_mul(t1[rows, :], nrm[rows, :],
                                             c_cos[rows, hs])
                        nc.vector.tensor_add(dst[rows, hs], t1[rows, :],
                                             t2[rows, :])
                        if is_kv:
                            nc.sync.dma_start(dst[0:64, hs],
                                              dst[64:128, hs])
                            if qh == 0:
                                nc.gpsimd.memset(v_aug[:].bitcast(f32),
                                                 1.0)
                            for tt in range(8 * qh, 8 * qh + 8):
                                ptr = psM.tile([128, 64], f32, tag="m",
                                               name=f"pt{qh}_{tt}")
                                nc.tensor.transpose(
                                    ptr[:],
                                    src[0:64, 128 * tt:128 * (tt + 1)],
                                    c_id[:])
                                nc.vector.tensor_copy(
                                    v_aug[:,
                                          (HD + 1) * tt:(HD + 1) * tt + HD],
                                    ptr[:])

            # consts for CDE (after AB's dma stream)
            nc.sync.dma_start(c_e2[:], e2)
            nc.sync.dma_start(c_mask[:], mask)
            for h in range(4):
                hr = slice(32 * h, 32 * h + 32)
                nc.sync.dma_start(c_wo0[hr, :], wo0[hr, :])
                nc.sync.dma_start(c_wo1[hr, :], wo1[hr, :])

            # ------- Fused phase C/D/E per q-chunk -------
            with tc.tile_pool(name="sbC", bufs=4) as sbC, \
                 tc.tile_pool(name="psS", bufs=2, space="PSUM") as psS, \
                 tc.tile_pool(name="psPV", bufs=2, space="PSUM") as psPV, \
                 tc.tile_pool(name="psO", bufs=2, space="PSUM") as psO:
                for qc in range(NQC):
                    qs = slice(512 * qc, 512 * qc + 512)
                    for hp, qr in ((0, qr0), (1, qr1)):
                        ppv_a = psPV.tile([65, 512], f32, tag="pv")
                        ppv_b = psPV.tile([65, 512], f32, tag="pv")
                        ntile = 4 * qc + 4
                        for t in range(ntile):
                            r = t - 4 * qc
                            off = max(0, r) * 128
                            qlo = 512 * qc + off
                            qlen = 512 * (qc + 1) - qlo
                            kc = slice(128 * t, 128 * (t + 1))
                            vs = slice((HD + 1) * t, (HD + 1) * t + HD + 1)
                            st = (t == 0)
                            sp = (t == ntile - 1)
                            ps_s = psS.tile([128, 1024], f32, tag="s")
                            nc.tensor.matmul(
                                ps_s[:, 0:qlen], krd[0:64, kc],
                                qr[0:64, qlo:qlo + qlen],
                                start=True, stop=True)
                            nc.tensor.matmul(
                                ps_s[:, 512:512 + qlen], krd[64:128, kc],
                                qr[64:128, qlo:qlo + qlen],
                                start=True, stop=True)
                            pt = sbC.tile([128, 1024], f32r, tag="pt")
                            if r >= 0:
                                nc.scalar.activation(
                                    pt[:, 0:512 + qlen],
                                    ps_s[:, 0:512 + qlen],
                                    AF.Exp, scale=SCALING)
                                nc.vector.tensor_mul(
                                    pt[:, 0:128], pt[:, 0:128], c_mask[:])
                                nc.vector.tensor_mul(
                                    pt[:, 512:640], pt[:, 512:640],
                                    c_mask[:])
                            else:
                                nc.scalar.activation(
                                    pt[:, 0:1024], ps_s[:, 0:1024],
                                    AF.Exp, scale=SCALING)
                            nc.tensor.matmul(
                                ppv_a[:, off:512], v_aug[:, vs],
                                pt[:, 0:qlen], start=st, stop=sp)
                            nc.tensor.matmul(
                                ppv_b[:, off:512], v_aug[:, vs],
                                pt[:, 512:512 + qlen], start=st, stop=sp)
                        for half, ppv in ((0, ppv_a), (1, ppv_b)):
                            stg = sbC.tile([65, 512], f32, tag="stg",
                                           bufs=3)
                            nc.vector.tensor_copy(stg[:], ppv[:])
                            nc.sync.dma_start(
                                attn_raw[hp][64 * half:64 * half + 64, qs],
                                stg[0:64, :])
                            nc.sync.dma_start(
                                l_sb[hp][half:half + 1, qs], stg[64:65, :])
                    # normalize this q-chunk: 1/l on DVE (keeps the
                    # ScalarE exp table resident through phase C)
                    for i in range(2):
                        rl = sbC.tile([2, 512], f32, tag="lnl", bufs=2,
                                      name=f"rl{i}")
                        nc.vector.reciprocal(rl[:], l_sb[i][:, qs])
                        rl_r = sbC.tile([2, 512], f32r, tag="rlr", bufs=2,
                                        name=f"rlr{i}")
                        nc.vector.tensor_copy(rl_r[:], rl[:])
                        pb = psO.tile([128, 512], f32, tag="o")
                        nc.tensor.matmul(pb[:], c_e2[:], rl_r[:],
                                         start=True, stop=True)
                        nc.vector.tensor_mul(
                            attn_raw[i][:, qs].bitcast(f32r),
                            attn_raw[i][:, qs], pb[:])
                    # o_proj rows + 256-row chunked reduce-scatter
                    for m in range(4 * qc, 4 * qc + 4):
                        ms = slice(128 * m, 128 * (m + 1))
                        ost = sbC.tile([128, S], rs_dt, tag="ost", bufs=2)
                        for n in range(4):
                            ns = slice(512 * n, 512 * n + 512)
                            po = psO.tile([128, 512], f32, tag="o")
                            nc.tensor.matmul(
                                po[:], attn_raw[0][:, ms].bitcast(f32r),
                                c_wo0[:, ns], start=True, stop=False)
                            nc.tensor.matmul(
                                po[:], attn_raw[1][:, ms].bitcast(f32r),
                                c_wo1[:, ns], start=False, stop=True)
                            nc.vector.tensor_copy(ost[:, ns], po[:])
                        nc.sync.dma_start(partial[ms, :], ost[:])
                        if m % 2 == 1:
                            ch = m // 2
                            nc.gpsimd.collective_compute(
                                "ReduceScatter",
                                mybir.AluOpType.add,
                                replica_groups=[list(range(N_CORES))],
                                ins=[partial[128 * (m - 1):128 * (m + 1),
                                             :].opt()],
                                outs=[rs_out[32 * ch:32 * ch + 32,
                                             :].opt()],
                            )
                            if RS_BF16:
                                stc = sbC.tile([32, S], rs_dt, tag="stc",
                                               bufs=2)
                                nc.sync.dma_start(
                                    stc[:], rs_out[32 * ch:32 * ch + 32, :])
                                stf = sbC.tile([32, S], f32, tag="stf",
                                               bufs=2)
                                nc.vector.tensor_copy(stf[:], stc[:])
                                nc.sync.dma_start(
                                    out_rs[32 * ch:32 * ch + 32, :],
                                    stf[:])
                            else:
                                nc.sync.dma_start(
                                    out_rs[32 * ch:32 * ch + 32, :],
                                    rs_out[32 * ch:32 * ch + 32,
                                           :].bitcast(f32))

    nc.compile()
    return nc


def _host_prep(hidden_states, position_ids, wq, wk, wv, wo, q_ln_w, k_ln_w):
    x = np.asarray(hidden_states, dtype=np.float32)[0]        # [S, HID]
    xT = np.ascontiguousarray(x.T)                            # [HID, S]
    pos = np.asarray(position_ids)[0].astype(np.float32)      # [S]
    inv = 1.0 / (ROPE_THETA ** (np.arange(0, HD, 2, dtype=np.float32) / HD))
    ang = pos[:, None] * inv[None, :]                         # [S, 32]
    emb = np.concatenate([ang, ang], axis=1)                  # [S, 64]
    cosT = np.cos(emb).T.astype(np.float32)                   # [64, S]
    sinT = np.sin(emb).T.astype(np.float32)
    ss = sinT.copy()
    ss[0:32] = -sinT[0:32]
    cos2 = np.tile(cosT, (2, 1))
    ss2 = np.tile(ss, (2, 1))

    e2 = np.zeros((2, 128), dtype=np.float32)
    e2[0, 0:64] = 1.0
    e2[1, 64:128] = 1.0
    ew_q = np.zeros((2, 128), dtype=np.float32)
    ew_q[0, 0:64] = q_ln_w
    ew_q[1, 64:128] = q_ln_w
    ew_k = np.zeros((2, 128), dtype=np.float32)
    ew_k[1, 64:128] = k_ln_w
    msk = (np.arange(128)[:, None] <= np.arange(128)[None, :]) \
        .astype(np.float32)
    ident = np.eye(64, dtype=np.float32)

    wq_ = np.asarray(wq, dtype=np.float32)
    wk_ = np.asarray(wk, dtype=np.float32)
    wv_ = np.asarray(wv, dtype=np.float32)
    wo_ = np.asarray(wo, dtype=np.float32)

    def pretile(w):  # [HID, 128] -> [128, HID] ktile-blocked
        return np.ascontiguousarray(
            w.reshape(NK, 128, 128).transpose(1, 0, 2).reshape(128, HID))

    in_maps = []
    for c in range(N_CORES):
        qcols = slice(256 * c, 256 * (c + 1))
        kvcols = slice(64 * c, 64 * (c + 1))
        wq_c = np.ascontiguousarray(wq_[:, qcols])
        wkv_c = np.concatenate([wv_[:, kvcols], wk_[:, kvcols]], axis=1)
        wo_c = np.ascontiguousarray(wo_[256 * c:256 * (c + 1), :])
        in_maps.append({
            "xT": xT,
            "wq0": pretile(wq_c[:, 0:128]),
            "wq1": pretile(wq_c[:, 128:256]),
            "wkv": pretile(wkv_c),
            "wo0": np.ascontiguousarray(wo_c[0:128, :]),
            "wo1": np.ascontiguousarray(wo_c[128:256, :]),
            "cos2": cos2,
            "ss2": ss2,
            "ew_q": ew_q,
            "ew_k": ew_k,
            "e2": e2,
            "e2t": np.ascontiguousarray(e2.T),
            "mask": msk,
            "ident": ident,
        })
    return in_maps


def kernel(hidden_states, position_ids, wq, wk, wv, wo, q_ln_w, k_ln_w):
    global _NC_CACHE, LAST_RESULTS
    if _NC_CACHE is None:
        _NC_CACHE = _build()
    nc = _NC_CACHE
    in_maps = _host_prep(hidden_states, position_ids, wq, wk, wv, wo,
                         q_ln_w, k_ln_w)
    res = bass_utils.run_bass_kernel_spmd(
        nc, in_maps, core_ids=list(range(N_CORES)))
    LAST_RESULTS = res
    out = np.empty((S, HID), dtype=np.float32)
    for c in range(N_CORES):
        o_c = res.results[c]["out_rs"]        # [256, 2048]
        for ch in range(8):
            out[256 * ch + 32 * c:256 * ch + 32 * c + 32, :] = \
                o_c[32 * ch:32 * ch + 32, :]
    return out.reshape(1, S, HID)



# revision 2
# speedup vs baseline: 1.0898x; 1.0898x over previous
"""GQA attention (B=1, S=2048, H=2048, 32 q-heads / 8 kv-heads, hd=64)
on 8 Trainium2 NeuronCores.

Sharding: tensor-parallel over heads. Core c owns q-heads 4c..4c+3 and
kv-head c: wq/wk/wv column shards. After attention, a 2-stage AllToAll
re-shards by sequence position (each core ends up with 2 blocks of 128
positions x all 32 heads), then each core runs the FULL o_proj
(complete wo replicated, bf16) on its position rows -- no reduction
needed, ~1MB total collective traffic instead of a 16MB ReduceScatter.

All matmuls bf16 (fp32 PSUM accumulation): ~4x the fp32r rate on HW.

Device program (per core):
  phase AB, pipelined per 1024-column half:
    qT/kT/vT projections (weights host-pretiled bf16), per-head RMSNorm
    (bf16 squares, ones-block matmul partition sums, Ln->Exp(-0.5x)
    rstd in bf16), RoPE via partition-shifted sbuf-sbuf DMA + 3 DVE
    ops (bf16), V transposed on TensorE into V_aug (bf16) with a ones
    column.
  phase CD per 512-wide q-chunk:
    scores^T [128 kpos, q] = kT-tile.T @ qT (bf16), 2 heads row-packed
    into one [128,1024] psum, P^T = exp(0.125 S^T) on ScalarE -> bf16,
    causal mask on the diagonal 128x128 block, attnT_aug [65, q] +=
    V_aug.T @ P^T (ones column -> l), normalize by 1/l (DVE recip)
    broadcast via ones-block matmul, stage normalized bf16 attnT
    slices into the AllToAll layout.
    AllToAll #0 after q-chunks 0-1 (pos 0:1024), #1 after 2-3.
  phase E: o_proj pos-tile 0 (from A2A#0) right after CD -- hides
    A2A#1 latency -- then pos-tile 1; fp32 output rows DMA'd out.
"""
import sys

sys.path.insert(0, "/opt/trn_rl_repo")

import numpy as np  # noqa: E402
import ml_dtypes  # noqa: E402
import concourse.bacc as bacc  # noqa: E402
import concourse.mybir as mybir  # noqa: E402
import concourse.tile as tile  # noqa: E402
from concourse import bass_utils  # noqa: E402

f32 = mybir.dt.float32
bf16 = mybir.dt.bfloat16
AF = mybir.ActivationFunctionType
BF = ml_dtypes.bfloat16

N_CORES = 8
S = 2048
HID = 2048
HD = 64
ROPE_THETA = 10000.0
RMS_EPS = 1e-6
SCALING = HD ** -0.5              # 0.125
NK = HID // 128                   # 16 contraction tiles
NQC = S // 512                    # 4 q chunks
NKT = S // 128                    # 16 kpos tiles

_NC_CACHE = None
LAST_RESULTS = None


def _build():
    nc = bacc.Bacc("TRN2", target_bir_lowering=False, debug=False,
                   num_devices=N_CORES)

    def din(name, shape, dt):
        return nc.dram_tensor(name, shape, dt, kind="ExternalInput").ap()

    xT = din("xT", [HID, S], bf16)
    # host-pretiled: row p, col block t = original rows 128t+p
    wq0 = din("wq0", [128, HID], bf16)
    wq1 = din("wq1", [128, HID], bf16)
    wkv = din("wkv", [128, HID], bf16)     # [wv | wk] columns pretiled
    wof = din("wof", [128, NK * S], bf16)  # full wo, block t = rows 128t+p
    cos2 = din("cos2", [128, S], bf16)
    ss2 = din("ss2", [128, S], bf16)
    ew_q = din("ew_q", [2, 128], bf16)
    ew_k = din("ew_k", [2, 128], bf16)
    e2t = din("e2t", [128, 2], bf16)
    sel4 = din("sel4", [4, 256], bf16)     # row g ones in cols 64g..64g+64
    mask = din("mask", [128, 128], bf16)
    ident = din("ident", [64, 64], bf16)

    out_o = nc.dram_tensor("out_o", [256, S], f32,
                           kind="ExternalOutput").ap()

    with tile.TileContext(nc) as tc:
        with tc.tile_pool(name="consts", bufs=1) as cp, \
             tc.tile_pool(name="dram", bufs=1, space="DRAM") as dp:
            c_wq0 = cp.tile([128, HID], bf16, tag="w")
            c_wq1 = cp.tile([128, HID], bf16, tag="w2")
            c_wkv = cp.tile([128, HID], bf16, tag="w3")
            c_wof = cp.tile([128, NK * S], bf16, tag="w4")
            c_cos = cp.tile([128, S], bf16, tag="c1")
            c_ss = cp.tile([128, S], bf16, tag="c2")
            c_ewq = cp.tile([2, 128], bf16, tag="c3")
            c_ewk = cp.tile([2, 128], bf16, tag="c4")
            c_e2t = cp.tile([128, 2], bf16, tag="c5t")
            c_sel = cp.tile([4, 256], bf16, tag="c5")
            c_mask = cp.tile([128, 128], bf16, tag="c6")
            c_id = cp.tile([64, 64], bf16, tag="c7")
            c_eps = cp.tile([2, 1], f32, tag="c8")

            # phase-A weights first (contiguous rows); wq0 complete
            # first so the first matmul unblocks early
            for dst_t, src_t in ((c_wq0, wq0), (c_wq1, wq1),
                                 (c_wkv, wkv)):
                for h in range(4):
                    hr = slice(32 * h, 32 * h + 32)
                    nc.sync.dma_start(dst_t[hr, :], src_t[hr, :])
            nc.vector.memset(c_eps[:], RMS_EPS)
            nc.sync.dma_start(c_e2t[:], e2t)
            nc.sync.dma_start(c_ewq[:], ew_q)
            nc.sync.dma_start(c_ewk[:], ew_k)
            nc.sync.dma_start(c_id[:], ident)
            nc.sync.dma_start(c_cos[:], cos2)
            nc.sync.dma_start(c_ss[:], ss2)
            nc.sync.dma_start(c_sel[:], sel4)
            nc.sync.dma_start(c_mask[:], mask)

            qr0 = cp.tile([128, S], bf16, tag="qr0")
            qr1 = cp.tile([128, S], bf16, tag="qr1")
            krd = cp.tile([128, S], bf16, tag="krd")
            v_aug = cp.tile([128, NKT * (HD + 1)], bf16, tag="vaug")
            l4 = cp.tile([4, S], f32, tag="l4")

            # AllToAll buffers: [2048, 128] bf16; rows 256j..256j+256 =
            # slot for rank j (rows within slot = local head-col,
            # cols = pos offset within the destination's 128-block)
            a2a_in = [dp.tile([16 * 128, 128], bf16, name=f"a2ai{b}")
                      for b in range(2)]
            a2a_out = [dp.tile([16 * 128, 128], bf16, name=f"a2ao{b}")
                       for b in range(2)]

            # ---- Phase A+B pipelined per 1024-col half ----
            with tc.tile_pool(name="xt", bufs=4) as xp, \
                 tc.tile_pool(name="sbB", bufs=2) as sbB, \
                 tc.tile_pool(name="psA", bufs=3, space="PSUM") as psA, \
                 tc.tile_pool(name="psM", bufs=2, space="PSUM") as psM:
                specs = [
                    ("kv", c_ewk, krd, True),
                    ("q0", c_ewq, qr0, False),
                    ("q1", c_ewq, qr1, False),
                ]
                for qh in range(2):
                    hs = slice(1024 * qh, 1024 * qh + 1024)
                    # --- A: projections for this half ---
                    pq = [psA.tile([128, 1024], f32, tag="pa",
                                   name=f"pa{qh}_{j}") for j in range(3)]
                    for t in range(NK):
                        xt = xp.tile([128, 1024], bf16, tag="xt")
                        for h in range(2):
                            hr = slice(64 * h, 64 * h + 64)
                            nc.sync.dma_start(
                                xt[hr, :],
                                xT[128 * t + 64 * h:128 * t + 64 * h + 64,
                                   hs])
                        st = (t == 0)
                        sp = (t == NK - 1)
                        tc_ = slice(128 * t, 128 * (t + 1))
                        for j, w in ((0, c_wq0), (1, c_wq1), (2, c_wkv)):
                            nc.tensor.matmul(pq[j][:, 0:512], w[:, tc_],
                                             xt[:, 0:512],
                                             start=st, stop=sp)
                            nc.tensor.matmul(pq[j][:, 512:1024], w[:, tc_],
                                             xt[:, 512:1024],
                                             start=st, stop=sp)
                    # psum -> sbuf bf16 copies
                    qkv = {}
                    for j, key in ((0, "q0"), (1, "q1"), (2, "kv")):
                        t_ = sbB.tile([128, 1024], bf16, tag=f"qkv{j}",
                                      bufs=2, name=f"qkv{qh}_{j}")
                        nc.vector.tensor_copy(t_[:], pq[j][:])
                        qkv[key] = t_

                    # --- B: norm + rope for the two 512-chunks ---
                    # stats: Ln batch then Exp batch (2 table switches)
                    lnvs = {}
                    for si, (key, ew, dst, is_kv) in enumerate(specs):
                        src = qkv[key]
                        sq = sbB.tile([128, 1024], bf16, tag="sq",
                                      bufs=2, name=f"sq{qh}_{si}")
                        nc.vector.tensor_mul(sq[:], src[:], src[:])
                        for u in range(2):
                            us = slice(512 * u, 512 * u + 512)
                            pss = psM.tile([2, 512], f32, tag="m",
                                           name=f"ss{qh}_{si}_{u}")
                            nc.tensor.matmul(pss[:], c_e2t[:], sq[:, us],
                                             start=True, stop=True)
                            lnv = sbB.tile([2, 512], f32, tag="lnv",
                                           bufs=6, name=f"lnv{qh}{si}{u}")
                            nc.scalar.activation(lnv[:], pss[:], AF.Ln,
                                                 scale=1.0 / HD,
                                                 bias=c_eps[:])
                            lnvs[(si, u)] = lnv
                    rstds = {}
                    for si in range(3):
                        for u in range(2):
                            rr = sbB.tile([2, 512], bf16, tag="rstdr",
                                          bufs=6, name=f"rr{qh}{si}{u}")
                            nc.scalar.activation(rr[:], lnvs[(si, u)][:],
                                                 AF.Exp, scale=-0.5)
                            rstds[(si, u)] = rr
                    for si, (key, ew, dst, is_kv) in enumerate(specs):
                        src = qkv[key]
                        rows = slice(64, 128) if is_kv else slice(0, 128)
                        nrm = sbB.tile([128, 1024], bf16, tag="nrm",
                                       bufs=2, name=f"nrm{qh}_{si}")
                        for u in range(2):
                            us = slice(512 * u, 512 * u + 512)
                            pb = psM.tile([128, 512], f32, tag="m",
                                          name=f"pb{qh}_{si}_{u}")
                            nc.tensor.matmul(pb[:], ew[:],
                                             rstds[(si, u)][:],
                                             start=True, stop=True)
                            nc.vector.tensor_mul(nrm[rows, us],
                                                 src[rows, us],
                                                 pb[rows, :])
                        # rope
                        sh = sbB.tile([128, 1024], bf16, tag="sh",
                                      bufs=2, name=f"sh{qh}_{si}")
                        if is_kv:
                            nc.sync.dma_start(sh[64:96, :], nrm[96:128, :])
                            nc.sync.dma_start(sh[96:128, :], nrm[64:96, :])
                        else:
                            nc.sync.dma_start(sh[0:32, :], nrm[32:64, :])
                            nc.sync.dma_start(sh[32:64, :], nrm[0:32, :])
                            nc.sync.dma_start(sh[64:96, :], nrm[96:128, :])
                            nc.sync.dma_start(sh[96:128, :], nrm[64:96, :])
                        t2 = sbB.tile([128, 1024], bf16, tag="sq",
                                      bufs=2, name=f"t2{qh}_{si}")
                        nc.vector.tensor_mul(t2[rows, :], sh[rows, :],
                                             c_ss[rows, hs])
                        t1 = sbB.tile([128, 1024], bf16, tag="sh",
                                      bufs=2, name=f"t1{qh}_{si}")
                        nc.vector.tensor_mul(t1[rows, :], nrm[rows, :],
                                             c_cos[rows, hs])
                        nc.vector.tensor_add(dst[rows, hs], t1[rows, :],
                                             t2[rows, :])
                        if is_kv:
                            nc.sync.dma_start(dst[0:64, hs],
                                              dst[64:128, hs])
                            if qh == 0:
                                nc.gpsimd.memset(v_aug[:], 1.0)
                            for tt in range(8 * qh, 8 * qh + 8):
                                ptr = psM.tile([128, 64], bf16, tag="m",
                                               name=f"pt{qh}_{tt}")
                                nc.tensor.transpose(
                                    ptr[:],
                                    src[0:64, 128 * (tt - 8 * qh):
                                        128 * (tt - 8 * qh) + 128],
                                    c_id[:])
                                nc.vector.tensor_copy(
                                    v_aug[:,
                                          (HD + 1) * tt:(HD + 1) * tt + HD],
                                    ptr[:])

            # wo full for phase E (after AB's dma stream)
            for h in range(16):
                hr = slice(8 * h, 8 * h + 8)
                nc.sync.dma_start(c_wof[hr, :], wof[hr, :])

            # ------- Fused phase C/D per q-chunk + staged A2A -------
            with tc.tile_pool(name="sbC", bufs=4) as sbC, \
                 tc.tile_pool(name="sbE", bufs=2) as sbE, \
                 tc.tile_pool(name="psS", bufs=2, space="PSUM") as psS, \
                 tc.tile_pool(name="psPV", bufs=2, space="PSUM") as psPV, \
                 tc.tile_pool(name="psO", bufs=2, space="PSUM") as psO:
                for qc in range(NQC):
                    qs = slice(512 * qc, 512 * qc + 512)
                    stgs = {}
                    for hp, qr in ((0, qr0), (1, qr1)):
                        ppv_a = psPV.tile([65, 512], f32, tag="pv",
                                          name=f"pva{qc}{hp}")
                        ppv_b = psPV.tile([65, 512], f32, tag="pv",
                                          name=f"pvb{qc}{hp}")
                        ntile = 4 * qc + 4
                        for t in range(ntile):
                            r = t - 4 * qc
                            off = max(0, r) * 128
                            qlo = 512 * qc + off
                            qlen = 512 * (qc + 1) - qlo
                            kc = slice(128 * t, 128 * (t + 1))
                            vs = slice((HD + 1) * t, (HD + 1) * t + HD + 1)
                            st = (t == 0)
                            sp = (t == ntile - 1)
                            ps_s = psS.tile([128, 1024], f32, tag="s")
                            nc.tensor.matmul(
                                ps_s[:, 0:qlen], krd[0:64, kc],
                                qr[0:64, qlo:qlo + qlen],
                                start=True, stop=True)
                            nc.tensor.matmul(
                                ps_s[:, 512:512 + qlen], krd[64:128, kc],
                                qr[64:128, qlo:qlo + qlen],
                                start=True, stop=True)
                            pt = sbC.tile([128, 1024], bf16, tag="pt")
                            if r >= 0:
                                nc.scalar.activation(
                                    pt[:, 0:512 + qlen],
                                    ps_s[:, 0:512 + qlen],
                                    AF.Exp, scale=SCALING)
                                nc.vector.tensor_mul(
                                    pt[:, 0:128], pt[:, 0:128], c_mask[:])
                                nc.vector.tensor_mul(
                                    pt[:, 512:640], pt[:, 512:640],
                                    c_mask[:])
                            else:
                                nc.scalar.activation(
                                    pt[:, 0:1024], ps_s[:, 0:1024],
                                    AF.Exp, scale=SCALING)
                            nc.tensor.matmul(
                                ppv_a[:, off:512], v_aug[:, vs],
                                pt[:, 0:qlen], start=st, stop=sp)
                            nc.tensor.matmul(
                                ppv_b[:, off:512], v_aug[:, vs],
                                pt[:, 512:512 + qlen], start=st, stop=sp)
                        for half, ppv in ((0, ppv_a), (1, ppv_b)):
                            g = 2 * hp + half
                            stg = sbC.tile([65, 512], f32, tag="stg",
                                           bufs=5, name=f"stg{qc}{g}")
                            nc.vector.tensor_copy(stg[:], ppv[:])
                            nc.sync.dma_start(l4[g:g + 1, qs],
                                              stg[64:65, :])
                            stgs[g] = stg
                    # normalize + stage into A2A layout
                    b = qc // 2
                    qh2 = qc % 2
                    rl = sbC.tile([4, 512], f32, tag="rl", bufs=2,
                                  name=f"rl{qc}")
                    nc.vector.reciprocal(rl[:], l4[:, qs])
                    rlb = sbC.tile([4, 512], bf16, tag="rlb", bufs=2,
                                   name=f"rlb{qc}")
                    nc.vector.tensor_copy(rlb[:], rl[:])
                    for g in range(4):
                        pbg = psPV.tile([64, 512], f32, tag="pv",
                                        name=f"pbg{qc}{g}")
                        nc.tensor.matmul(pbg[:], c_sel[:, 64 * g:64 * g + 64],
                                         rlb[:], start=True, stop=True)
                        an = sbC.tile([64, 512], bf16, tag="an", bufs=4,
                                      name=f"an{qc}{g}")
                        nc.vector.tensor_mul(an[:], stgs[g][0:64, :],
                                             pbg[:])
                        for jj in range(4):
                            j = 4 * qh2 + jj
                            ro = 256 * j + 64 * g
                            nc.sync.dma_start(
                                a2a_in[b][ro:ro + 64, :],
                                an[:, 128 * jj:128 * jj + 128])
                    if qc % 2 == 1:
                        nc.gpsimd.collective_compute(
                            "AllToAll",
                            mybir.AluOpType.bypass,
                            replica_groups=[list(range(N_CORES))],
                            ins=[a2a_in[b][:]],
                            outs=[a2a_out[b][:]],
                        )

                # ------- Phase E: o_proj per pos-tile -------
                for b in range(2):
                    att = sbE.tile([128, NK * 128], bf16, tag="att",
                                   name=f"att{b}")
                    for t in range(NK):
                        nc.sync.dma_start(
                            att[:, 128 * t:128 * t + 128],
                            a2a_out[b][128 * t:128 * t + 128, :])
                    ost = sbE.tile([128, S], f32, tag="ost",
                                   name=f"ost{b}")
                    for p in range(2):
                        po = [psO.tile([128, 512], f32, tag="o",
                                       name=f"po{b}{p}{i}")
                              for i in range(2)]
                        for t in range(NK):
                            at = att[:, 128 * t:128 * t + 128]
                            st = (t == 0)
                            sp = (t == NK - 1)
                            for i in range(2):
                                ws = slice(2048 * t + 1024 * p + 512 * i,
                                           2048 * t + 1024 * p + 512 * i
                                           + 512)
                                nc.tensor.matmul(po[i][:], at,
                                                 c_wof[:, ws],
                                                 start=st, stop=sp)
                        for i in range(2):
                            nc.vector.tensor_copy(
                                ost[:, 1024 * p + 512 * i:
                                    1024 * p + 512 * i + 512], po[i][:])
                    for i in range(4):
                        nc.sync.dma_start(
                            out_o[128 * b:128 * b + 128,
                                  512 * i:512 * i + 512],
                            ost[:, 512 * i:512 * i + 512])

    nc.compile()
    return nc


def _host_prep(hidden_states, position_ids, wq, wk, wv, wo, q_ln_w, k_ln_w):
    x = np.asarray(hidden_states, dtype=np.float32)[0]        # [S, HID]
    xT = np.ascontiguousarray(x.T).astype(BF)                 # [HID, S]
    pos = np.asarray(position_ids)[0].astype(np.float32)      # [S]
    inv = 1.0 / (ROPE_THETA ** (np.arange(0, HD, 2, dtype=np.float32) / HD))
    ang = pos[:, None] * inv[None, :]                         # [S, 32]
    emb = np.concatenate([ang, ang], axis=1)                  # [S, 64]
    cosT = np.cos(emb).T.astype(np.float32)                   # [64, S]
    sinT = np.sin(emb).T.astype(np.float32)
    ss = sinT.copy()
    ss[0:32] = -sinT[0:32]
    cos2 = np.tile(cosT, (2, 1)).astype(BF)
    ss2 = np.tile(ss, (2, 1)).astype(BF)

    ew_q = np.zeros((2, 128), dtype=np.float32)
    ew_q[0, 0:64] = q_ln_w
    ew_q[1, 64:128] = q_ln_w
    ew_k = np.zeros((2, 128), dtype=np.float32)
    ew_k[1, 64:128] = k_ln_w
    e2t = np.zeros((128, 2), dtype=np.float32)
    e2t[0:64, 0] = 1.0
    e2t[64:128, 1] = 1.0
    sel4 = np.zeros((4, 256), dtype=np.float32)
    for g in range(4):
        sel4[g, 64 * g:64 * g + 64] = 1.0
    msk = (np.arange(128)[:, None] <= np.arange(128)[None, :]) \
        .astype(np.float32)
    ident = np.eye(64, dtype=np.float32)

    wq_ = np.asarray(wq, dtype=np.float32)
    wk_ = np.asarray(wk, dtype=np.float32)
    wv_ = np.asarray(wv, dtype=np.float32)
    wo_ = np.asarray(wo, dtype=np.float32)

    def pretile(w):  # [HID, 128] -> [128, HID] ktile-blocked
        return np.ascontiguousarray(
            w.reshape(NK, 128, 128).transpose(1, 0, 2).reshape(128, HID))

    # full wo pretiled: [2048, 2048] -> [128, 16*2048], block t = rows
    # 128t..128t+128
    wof = np.ascontiguousarray(
        wo_.reshape(NK, 128, S).transpose(1, 0, 2).reshape(128, NK * S)
    ).astype(BF)

    in_maps = []
    for c in range(N_CORES):
        qcols = slice(256 * c, 256 * (c + 1))
        kvcols = slice(64 * c, 64 * (c + 1))
        wq_c = np.ascontiguousarray(wq_[:, qcols])
        wkv_c = np.concatenate([wv_[:, kvcols], wk_[:, kvcols]], axis=1)
        in_maps.append({
            "xT": xT,
            "wq0": pretile(wq_c[:, 0:128]).astype(BF),
            "wq1": pretile(wq_c[:, 128:256]).astype(BF),
            "wkv": pretile(wkv_c).astype(BF),
            "wof": wof,
            "cos2": cos2,
            "ss2": ss2,
            "ew_q": ew_q.astype(BF),
            "ew_k": ew_k.astype(BF),
            "e2t": e2t.astype(BF),
            "sel4": sel4.astype(BF),
            "mask": msk.astype(BF),
            "ident": ident.astype(BF),
        })
    return in_maps


def kernel(hidden_states, position_ids, wq, wk, wv, wo, q_ln_w, k_ln_w):
    global _NC_CACHE, LAST_RESULTS
    if _NC_CACHE is None:
        _NC_CACHE = _build()
    nc = _NC_CACHE
    in_maps = _host_prep(hidden_states, position_ids, wq, wk, wv, wo,
                         q_ln_w, k_ln_w)
    res = bass_utils.run_bass_kernel_spmd(
        nc, in_maps, core_ids=list(range(N_CORES)))
    LAST_RESULTS = res
    out = np.empty((S, HID), dtype=np.float32)
    for c in range(N_CORES):
        o_c = res.results[c]["out_o"]         # [256, 2048]
        out[128 * c:128 * c + 128, :] = o_c[0:128, :]
        out[1024 + 128 * c:1024 + 128 * c + 128, :] = o_c[128:256, :]
    return out.reshape(1, S, HID)


# revision 10
# speedup vs baseline: 1.3963x; 1.2812x over previous
"""GQA attention (B=1, S=2048, H=2048, 32 q-heads / 8 kv-heads, hd=64)
on 8 Trainium2 NeuronCores.

Sharding: tensor-parallel over heads. Core c owns q-heads 4c..4c+3 and
kv-head c: wq/wk/wv column shards. After attention, a 2-stage AllToAll
re-shards by sequence position (each core ends up with 2 blocks of 128
positions x all 32 heads), then each core runs the FULL o_proj
(complete wo replicated, bf16) on its position rows -- no reduction
needed, ~1MB total collective traffic instead of a 16MB ReduceScatter.

All matmuls bf16 (fp32 PSUM accumulation): ~4x the fp32r rate on HW.

Device program (per core):
  phase AB, pipelined per 1024-column half:
    qT/kT/vT projections (weights host-pretiled bf16), per-head RMSNorm
    (bf16 squares, ones-block matmul partition sums, Ln->Exp(-0.5x)
    rstd in bf16), RoPE via partition-shifted sbuf-sbuf DMA + 3 DVE
    ops (bf16), V transposed on TensorE into V_aug (bf16) with a ones
    column.
  phase CD per 512-wide q-chunk:
    scores^T [128 kpos, q] = kT-tile.T @ qT (bf16), 2 heads row-packed
    into one [128,1024] psum, P^T = exp(0.125 S^T) on ScalarE -> bf16,
    causal mask on the diagonal 128x128 block, attnT_aug [65, q] +=
    V_aug.T @ P^T (ones column -> l), normalize by 1/l (DVE recip)
    broadcast via ones-block matmul, stage normalized bf16 attnT
    slices into the AllToAll layout.
    AllToAll #0 after q-chunks 0-1 (pos 0:1024), #1 after 2-3.
  phase E: o_proj pos-tile 0 (from A2A#0) right after CD -- hides
    A2A#1 latency -- then pos-tile 1; fp32 output rows DMA'd out.
"""
import sys

sys.path.insert(0, "/opt/trn_rl_repo")

import numpy as np  # noqa: E402
import ml_dtypes  # noqa: E402
import concourse.bacc as bacc  # noqa: E402
import concourse.mybir as mybir  # noqa: E402
import concourse.tile as tile  # noqa: E402
from concourse import bass_utils  # noqa: E402

f32 = mybir.dt.float32
bf16 = mybir.dt.bfloat16
AF = mybir.ActivationFunctionType
BF = ml_dtypes.bfloat16

N_CORES = 8
S = 2048
HID = 2048
HD = 64
ROPE_THETA = 10000.0
RMS_EPS = 1e-6
SCALING = HD ** -0.5              # 0.125
NK = HID // 128                   # 16 contraction tiles
NQC = S // 512                    # 4 q chunks
NKT = S // 128                    # 16 kpos tiles

_NC_CACHE = None
LAST_RESULTS = None


def _build():
    nc = bacc.Bacc("TRN2", target_bir_lowering=False, debug=False,
                   num_devices=N_CORES)

    def din(name, shape, dt):
        return nc.dram_tensor(name, shape, dt, kind="ExternalInput").ap()

    xT = din("xT", [HID, S], bf16)
    # host-pretiled: row p, col block t = original rows 128t+p
    wq0 = din("wq0", [128, HID], bf16)
    wq1 = din("wq1", [128, HID], bf16)
    wkv = din("wkv", [128, HID], bf16)     # [wv | wk] columns pretiled
    wof = din("wof", [128, NK * S], bf16)  # full wo, block t = rows 128t+p
    cos2 = din("cos2", [128, S], bf16)
    ss2 = din("ss2", [128, S], bf16)
    ew_q = din("ew_q", [2, 128], bf16)
    ew_k = din("ew_k", [2, 128], bf16)
    e2t = din("e2t", [128, 2], bf16)
    sel4 = din("sel4", [4, 256], bf16)     # row g ones in cols 64g..64g+64
    mask = din("mask", [128, 128], bf16)
    ident = din("ident", [64, 64], bf16)

    out_o = nc.dram_tensor("out_o", [256, S], f32,
                           kind="ExternalOutput").ap()

    with tile.TileContext(nc) as tc:
        with tc.tile_pool(name="consts", bufs=1) as cp, \
             tc.tile_pool(name="dram", bufs=1, space="DRAM") as dp:
            c_wq0 = cp.tile([128, HID], bf16, tag="w")
            c_wq1 = cp.tile([128, HID], bf16, tag="w2")
            c_wkv = cp.tile([128, HID], bf16, tag="w3")
            c_cos = cp.tile([128, S], bf16, tag="c1")
            c_ss = cp.tile([128, S], bf16, tag="c2")
            c_ewq = cp.tile([2, 128], bf16, tag="c3")
            c_ewk = cp.tile([2, 128], bf16, tag="c4")
            c_e2t = cp.tile([128, 2], bf16, tag="c5t")
            c_sel = cp.tile([4, 256], bf16, tag="c5")
            c_mask = cp.tile([128, 128], bf16, tag="c6")
            c_id = cp.tile([64, 64], bf16, tag="c7")
            c_eps = cp.tile([2, 1], f32, tag="c8")

            # phase-A weights first (contiguous rows); wq0 complete
            # first so the first matmul unblocks early
            for dst_t, src_t in ((c_wq0, wq0), (c_wq1, wq1),
                                 (c_wkv, wkv)):
                for h in range(4):
                    hr = slice(32 * h, 32 * h + 32)
                    nc.sync.dma_start(dst_t[hr, :], src_t[hr, :])
            nc.vector.memset(c_eps[:], RMS_EPS)
            nc.sync.dma_start(c_e2t[:], e2t)
            nc.sync.dma_start(c_ewq[:], ew_q)
            nc.sync.dma_start(c_ewk[:], ew_k)
            nc.sync.dma_start(c_id[:], ident)
            for h in range(4):
                hr = slice(32 * h, 32 * h + 32)
                nc.sync.dma_start(c_cos[hr, :], cos2[hr, :])
                nc.sync.dma_start(c_ss[hr, :], ss2[hr, :])
            nc.sync.dma_start(c_sel[:], sel4)
            nc.sync.dma_start(c_mask[:], mask)

            qr0 = cp.tile([128, S], bf16, tag="qr0")
            qr1 = cp.tile([128, S], bf16, tag="qr1")
            krd = cp.tile([128, S], bf16, tag="krd")
            v_aug = cp.tile([128, NKT * (HD + 1)], bf16, tag="vaug")
            l4 = cp.tile([4, S], f32, tag="l4")

            # AllToAll buffers: [2048, 128] bf16; rows 256j..256j+256 =
            # slot for rank j (rows within slot = local head-col,
            # cols = pos offset within the destination's 128-block)
            a2a_in = [dp.tile([16 * 128, 128], bf16, name=f"a2ai{b}")
                      for b in range(2)]
            a2a_out = [dp.tile([16 * 128, 128], bf16, name=f"a2ao{b}")
                       for b in range(2)]

            # ---- Phase A+B pipelined per 1024-col half ----
            # full-row xT tiles resident through AB (4KB DMA packets;
            # loaded once in half 0, reused in half 1)
            with tc.tile_pool(name="xt", bufs=16) as xp, \
                 tc.tile_pool(name="sbB", bufs=2) as sbB, \
                 tc.tile_pool(name="psA", bufs=3, space="PSUM") as psA, \
                 tc.tile_pool(name="psM", bufs=2, space="PSUM") as psM:
                specs = [
                    ("kv", c_ewk, krd, True),
                    ("q0", c_ewq, qr0, False),
                    ("q1", c_ewq, qr1, False),
                ]
                xts = []
                for qh in range(2):
                    hs = slice(1024 * qh, 1024 * qh + 1024)
                    # --- A: projections for this half ---
                    pq = [psA.tile([128, 1024], f32, tag="pa",
                                   name=f"pa{qh}_{j}") for j in range(3)]
                    for t in range(NK):
                        if qh == 0:
                            xt = xp.tile([128, S], bf16, tag="xt",
                                         name=f"xt{t}")
                            for h in range(4):
                                hr = slice(32 * h, 32 * h + 32)
                                nc.sync.dma_start(
                                    xt[hr, :],
                                    xT[128 * t + 32 * h:
                                       128 * t + 32 * h + 32, :])
                            xts.append(xt)
                        else:
                            xt = xts[t]
                        st = (t == 0)
                        sp = (t == NK - 1)
                        tc_ = slice(128 * t, 128 * (t + 1))
                        for j, w in ((0, c_wq0), (1, c_wq1), (2, c_wkv)):
                            nc.tensor.matmul(pq[j][:, 0:512], w[:, tc_],
                                             xt[:, 1024 * qh:
                                                1024 * qh + 512],
                                             start=st, stop=sp)
                            nc.tensor.matmul(pq[j][:, 512:1024], w[:, tc_],
                                             xt[:, 1024 * qh + 512:
                                                1024 * qh + 1024],
                                             start=st, stop=sp)
                    # psum -> sbuf bf16 copies
                    qkv = {}
                    for j, key in ((0, "q0"), (1, "q1"), (2, "kv")):
                        t_ = sbB.tile([128, 1024], bf16, tag=f"qkv{j}",
                                      bufs=2, name=f"qkv{qh}_{j}")
                        nc.vector.tensor_copy(t_[:], pq[j][:])
                        qkv[key] = t_

                    # --- B: norm + rope for the two 512-chunks ---
                    # stats: Ln batch then Exp batch (2 table switches)
                    lnvs = {}
                    for si, (key, ew, dst, is_kv) in enumerate(specs):
                        src = qkv[key]
                        sq = sbB.tile([128, 1024], bf16, tag="sq",
                                      bufs=2, name=f"sq{qh}_{si}")
                        nc.vector.tensor_mul(sq[:], src[:], src[:])
                        for u in range(2):
                            us = slice(512 * u, 512 * u + 512)
                            pss = psM.tile([2, 512], f32, tag="m",
                                           name=f"ss{qh}_{si}_{u}")
                            nc.tensor.matmul(pss[:], c_e2t[:], sq[:, us],
                                             start=True, stop=True)
                            lnv = sbB.tile([2, 512], f32, tag="lnv",
                                           bufs=6, name=f"lnv{qh}{si}{u}")
                            nc.scalar.activation(lnv[:], pss[:], AF.Ln,
                                                 scale=1.0 / HD,
                                                 bias=c_eps[:])
                            lnvs[(si, u)] = lnv
                    rstds = {}
                    for si in range(3):
                        for u in range(2):
                            rr = sbB.tile([2, 512], bf16, tag="rstdr",
                                          bufs=6, name=f"rr{qh}{si}{u}")
                            nc.scalar.activation(rr[:], lnvs[(si, u)][:],
                                                 AF.Exp, scale=-0.5)
                            rstds[(si, u)] = rr
                    for si, (key, ew, dst, is_kv) in enumerate(specs):
                        src = qkv[key]
                        rows = slice(64, 128) if is_kv else slice(0, 128)
                        nrm = sbB.tile([128, 1024], bf16, tag="nrm",
                                       bufs=2, name=f"nrm{qh}_{si}")
                        for u in range(2):
                            us = slice(512 * u, 512 * u + 512)
                            pb = psM.tile([128, 512], f32, tag="m",
                                          name=f"pb{qh}_{si}_{u}")
                            nc.tensor.matmul(pb[:], ew[:],
                                             rstds[(si, u)][:],
                                             start=True, stop=True)
                            nc.vector.tensor_mul(nrm[rows, us],
                                                 src[rows, us],
                                                 pb[rows, :])
                        # rope
                        sh = sbB.tile([128, 1024], bf16, tag="sh",
                                      bufs=2, name=f"sh{qh}_{si}")
                        if is_kv:
                            nc.sync.dma_start(sh[64:96, :], nrm[96:128, :])
                            nc.sync.dma_start(sh[96:128, :], nrm[64:96, :])
                        else:
                            nc.sync.dma_start(sh[0:32, :], nrm[32:64, :])
                            nc.sync.dma_start(sh[32:64, :], nrm[0:32, :])
                            nc.sync.dma_start(sh[64:96, :], nrm[96:128, :])
                            nc.sync.dma_start(sh[96:128, :], nrm[64:96, :])
                        t2 = sbB.tile([128, 1024], bf16, tag="sq",
                                      bufs=2, name=f"t2{qh}_{si}")
                        nc.vector.tensor_mul(t2[rows, :], sh[rows, :],
                                             c_ss[rows, hs])
                        t1 = sbB.tile([128, 1024], bf16, tag="sh",
                                      bufs=2, name=f"t1{qh}_{si}")
                        nc.vector.tensor_mul(t1[rows, :], nrm[rows, :],
                                             c_cos[rows, hs])
                        nc.vector.tensor_add(dst[rows, hs], t1[rows, :],
                                             t2[rows, :])
                        if is_kv:
                            nc.sync.dma_start(dst[0:64, hs],
                                              dst[64:128, hs])
                            if qh == 0:
                                nc.gpsimd.memset(v_aug[:], 1.0)
                            for tt in range(8 * qh, 8 * qh + 8):
                                ptr = psM.tile([128, 64], bf16, tag="m",
                                               name=f"pt{qh}_{tt}")
                                nc.tensor.transpose(
                                    ptr[:],
                                    src[0:64, 128 * (tt - 8 * qh):
                                        128 * (tt - 8 * qh) + 128],
                                    c_id[:])
                                nc.vector.tensor_copy(
                                    v_aug[:,
                                          (HD + 1) * tt:(HD + 1) * tt + HD],
                                    ptr[:])

            # ------- Fused phase C/D per q-chunk + staged A2A -------
            # wof pool opens after AB closed: reuses the xT region; 4KB
            # packets, emitted after all AB DMAs so queues drain in order
            with tc.tile_pool(name="wof", bufs=1) as wp, \
                 tc.tile_pool(name="sbC", bufs=4) as sbC, \
                 tc.tile_pool(name="sbE", bufs=2) as sbE, \
                 tc.tile_pool(name="psS", bufs=2, space="PSUM") as psS, \
                 tc.tile_pool(name="psPV", bufs=2, space="PSUM") as psPV, \
                 tc.tile_pool(name="psO", bufs=2, space="PSUM") as psO:
                c_wof = wp.tile([128, NK * S], bf16, tag="w4")
                for ch in range(NK):
                    cs = slice(2048 * ch, 2048 * ch + 2048)
                    nc.sync.dma_start(c_wof[:, cs], wof[:, cs])
                for qc in range(NQC):
                    qs = slice(512 * qc, 512 * qc + 512)
                    stgs = {}
                    for hp, qr in ((0, qr0), (1, qr1)):
                        ppv_a = psPV.tile([65, 512], f32, tag="pv",
                                          name=f"pva{qc}{hp}")
                        ppv_b = psPV.tile([65, 512], f32, tag="pv",
                                          name=f"pvb{qc}{hp}")
                        ntile = 4 * qc + 4
                        for t in range(ntile):
                            r = t - 4 * qc
                            off = max(0, r) * 128
                            qlo = 512 * qc + off
                            qlen = 512 * (qc + 1) - qlo
                            kc = slice(128 * t, 128 * (t + 1))
                            vs = slice((HD + 1) * t, (HD + 1) * t + HD + 1)
                            st = (t == 0)
                            sp = (t == ntile - 1)
                            ps_s = psS.tile([128, 1024], f32, tag="s")
                            nc.tensor.matmul(
                                ps_s[:, 0:qlen], krd[0:64, kc],
                                qr[0:64, qlo:qlo + qlen],
                                start=True, stop=True)
                            nc.tensor.matmul(
                                ps_s[:, 512:512 + qlen], krd[64:128, kc],
                                qr[64:128, qlo:qlo + qlen],
                                start=True, stop=True)
                            pt = sbC.tile([128, 1024], bf16, tag="pt")
                            if r >= 0:
                                nc.scalar.activation(
                                    pt[:, 0:512 + qlen],
                                    ps_s[:, 0:512 + qlen],
                                    AF.Exp, scale=SCALING)
                                nc.vector.tensor_mul(
                                    pt[:, 0:128], pt[:, 0:128], c_mask[:])
                                nc.vector.tensor_mul(
                                    pt[:, 512:640], pt[:, 512:640],
                                    c_mask[:])
                            else:
                                nc.scalar.activation(
                                    pt[:, 0:1024], ps_s[:, 0:1024],
                                    AF.Exp, scale=SCALING)
                            nc.tensor.matmul(
                                ppv_a[:, off:512], v_aug[:, vs],
                                pt[:, 0:qlen], start=st, stop=sp)
                            nc.tensor.matmul(
                                ppv_b[:, off:512], v_aug[:, vs],
                                pt[:, 512:512 + qlen], start=st, stop=sp)
                        for half, ppv in ((0, ppv_a), (1, ppv_b)):
                            g = 2 * hp + half
                            stg = sbC.tile([65, 512], f32, tag="stg",
                                           bufs=5, name=f"stg{qc}{g}")
                            nc.vector.tensor_copy(stg[:], ppv[:])
                            nc.gpsimd.dma_start(l4[g:g + 1, qs],
                                                stg[64:65, :])
                            stgs[g] = stg
                    # normalize + stage into A2A layout
                    b = qc // 2
                    qh2 = qc % 2
                    rl = sbC.tile([4, 512], f32, tag="rl", bufs=2,
                                  name=f"rl{qc}")
                    nc.vector.reciprocal(rl[:], l4[:, qs])
                    rlb = sbC.tile([4, 512], bf16, tag="rlb", bufs=2,
                                   name=f"rlb{qc}")
                    nc.vector.tensor_copy(rlb[:], rl[:])
                    for g in range(4):
                        pbg = psPV.tile([64, 512], f32, tag="pv",
                                        name=f"pbg{qc}{g}")
                        nc.tensor.matmul(pbg[:], c_sel[:, 64 * g:64 * g + 64],
                                         rlb[:], start=True, stop=True)
                        an = sbC.tile([64, 512], bf16, tag="an", bufs=4,
                                      name=f"an{qc}{g}")
                        nc.vector.tensor_mul(an[:], stgs[g][0:64, :],
                                             pbg[:])
                        for jj in range(4):
                            j = 4 * qh2 + jj
                            ro = 256 * j + 64 * g
                            nc.gpsimd.dma_start(
                                a2a_in[b][ro:ro + 64, :],
                                an[:, 128 * jj:128 * jj + 128])
                    if qc % 2 == 1:
                        nc.gpsimd.collective_compute(
                            "AllToAll",
                            mybir.AluOpType.bypass,
                            replica_groups=[list(range(N_CORES))],
                            ins=[a2a_in[b][:]],
                            outs=[a2a_out[b][:]],
                        )

                # ------- Phase E: o_proj per pos-tile -------
                for b in range(2):
                    att = sbE.tile([128, NK * 128], bf16, tag="att",
                                   name=f"att{b}")
                    for t in range(NK):
                        nc.sync.dma_start(
                            att[:, 128 * t:128 * t + 128],
                            a2a_out[b][128 * t:128 * t + 128, :])
                    ost = sbE.tile([128, S], f32, tag="ost",
                                   name=f"ost{b}")
                    for p in range(2):
                        po = [psO.tile([128, 512], f32, tag="o",
                                       name=f"po{b}{p}{i}")
                              for i in range(2)]
                        for t in range(NK):
                            at = att[:, 128 * t:128 * t + 128]
                            st = (t == 0)
                            sp = (t == NK - 1)
                            for i in range(2):
                                ws = slice(2048 * t + 1024 * p + 512 * i,
                                           2048 * t + 1024 * p + 512 * i
                                           + 512)
                                nc.tensor.matmul(po[i][:], at,
                                                 c_wof[:, ws],
                                                 start=st, stop=sp)
                        for i in range(2):
                            nc.vector.tensor_copy(
                                ost[:, 1024 * p + 512 * i:
                                    1024 * p + 512 * i + 512], po[i][:])
                        # out rows in [32, 4KB] pieces across queues,
                        # issued per 1024-col half to overlap the tail
                        for rg in range(4):
                            rs_ = slice(32 * rg, 32 * rg + 32)
                            nc.sync.dma_start(
                                out_o[128 * b + 32 * rg:
                                      128 * b + 32 * rg + 32,
                                      1024 * p:1024 * p + 1024],
                                ost[rs_, 1024 * p:1024 * p + 1024])

    nc.compile()
    return nc


def _host_prep(hidden_states, position_ids, wq, wk, wv, wo, q_ln_w, k_ln_w):
    x = np.asarray(hidden_states, dtype=np.float32)[0]        # [S, HID]
    xT = np.ascontiguousarray(x.T).astype(BF)                 # [HID, S]
    pos = np.asarray(position_ids)[0].astype(np.float32)      # [S]
    inv = 1.0 / (ROPE_THETA ** (np.arange(0, HD, 2, dtype=np.float32) / HD))
    ang = pos[:, None] * inv[None, :]                         # [S, 32]
    emb = np.concatenate([ang, ang], axis=1)                  # [S, 64]
    cosT = np.cos(emb).T.astype(np.float32)                   # [64, S]
    sinT = np.sin(emb).T.astype(np.float32)
    ss = sinT.copy()
    ss[0:32] = -sinT[0:32]
    cos2 = np.tile(cosT, (2, 1)).astype(BF)
    ss2 = np.tile(ss, (2, 1)).astype(BF)

    ew_q = np.zeros((2, 128), dtype=np.float32)
    ew_q[0, 0:64] = q_ln_w
    ew_q[1, 64:128] = q_ln_w
    ew_k = np.zeros((2, 128), dtype=np.float32)
    ew_k[1, 64:128] = k_ln_w
    e2t = np.zeros((128, 2), dtype=np.float32)
    e2t[0:64, 0] = 1.0
    e2t[64:128, 1] = 1.0
    sel4 = np.zeros((4, 256), dtype=np.float32)
    for g in range(4):
        sel4[g, 64 * g:64 * g + 64] = 1.0
    msk = (np.arange(128)[:, None] <= np.arange(128)[None, :]) \
        .astype(np.float32)
    ident = np.eye(64, dtype=np.float32)

    wq_ = np.asarray(wq, dtype=np.float32)
    wk_ = np.asarray(wk, dtype=np.float32)
    wv_ = np.asarray(wv, dtype=np.float32)
    wo_ = np.asarray(wo, dtype=np.float32)

    def pretile(w):  # [HID, 128] -> [128, HID] ktile-blocked
        return np.ascontiguousarray(
            w.reshape(NK, 128, 128).transpose(1, 0, 2).reshape(128, HID))

    # full wo pretiled: [2048, 2048] -> [128, 16*2048], block t = rows
    # 128t..128t+128
    wof = np.ascontiguousarray(
        wo_.reshape(NK, 128, S).transpose(1, 0, 2).reshape(128, NK * S)
    ).astype(BF)

    in_maps = []
    for c in range(N_CORES):
        qcols = slice(256 * c, 256 * (c + 1))
        kvcols = slice(64 * c, 64 * (c + 1))
        wq_c = np.ascontiguousarray(wq_[:, qcols])
        wkv_c = np.concatenate([wv_[:, kvcols], wk_[:, kvcols]], axis=1)
        in_maps.append({
            "xT": xT,
            "wq0": pretile(wq_c[:, 0:128]).astype(BF),
            "wq1": pretile(wq_c[:, 128:256]).astype(BF),
            "wkv": pretile(wkv_c).astype(BF),
            "wof": wof,
            "cos2": cos2,
            "ss2": ss2,
            "ew_q": ew_q.astype(BF),
            "ew_k": ew_k.astype(BF),
            "e2t": e2t.astype(BF),
            "sel4": sel4.astype(BF),
            "mask": msk.astype(BF),
            "ident": ident.astype(BF),
        })
    return in_maps


def kernel(hidden_states, position_ids, wq, wk, wv, wo, q_ln_w, k_ln_w):
    global _NC_CACHE, LAST_RESULTS
    if _NC_CACHE is None:
        _NC_CACHE = _build()
    nc = _NC_CACHE
    in_maps = _host_prep(hidden_states, position_ids, wq, wk, wv, wo,
                         q_ln_w, k_ln_w)
    res = bass_utils.run_bass_kernel_spmd(
        nc, in_maps, core_ids=list(range(N_CORES)))
    LAST_RESULTS = res
    out = np.empty((S, HID), dtype=np.float32)
    for c in range(N_CORES):
        o_c = res.results[c]["out_o"]         # [256, 2048]
        out[128 * c:128 * c + 128, :] = o_c[0:128, :]
        out[1024 + 128 * c:1024 + 128 * c + 128, :] = o_c[128:256, :]
    return out.reshape(1, S, HID)
